# revision 13
# baseline (speedup 1.0000x reference)
"""Quantized matmul (uint4 groupwise dequant) on 8 Trainium2 NeuronCores.

Computes out = a_f32 @ W where W[k, n] = (q[k, n] - zeros[k//128, n]) * scales[k//128, n].

Sharding: 2-D tensor-parallel (4 m-groups x 2 n-groups). Each core gets
M_L = 1024 rows of `a` and N_L = 2048 output columns of q/scales/zeros.
This is the min-DMA sharding (24.4 MB/core vs 42 MB for pure-N TP).

Algorithm (hybrid fp8 DoubleRow + fp16, all arithmetic on device):
  W = Wc + rep(mu), with Wc[k,n] = (q[k,n] - 7.5) * s[g,n]  (zero-mean-ish)
  and mu[g,n] = (7.5 - z[g,n]) * s[g,n].
  out = a @ Wc + A @ mu, where A[m,g] = sum_{k in group g} a[m,k].

  - ktiles 0..NFP8-1 of Wc go to fp8e4; a goes to fp8e4; those contractions
    run with perf_mode=DoubleRow (2 k-planes per pass). Centering by 7.5
    (not z) keeps E[Wc^2] low enough that the fp8 rounding noise of both
    operands stays inside the 2e-2 rel-err budget.
  - Remaining ktiles stay fp16 (exact inputs) to claw back precision.
  - The rank-32 correction A @ mu runs in fp16. A is built on the PE with
    one-hot selector matmuls (exact fp16 a), 4-way column-tiled so four
    mtiles' A columns compute concurrently.

Encoding trick: the host ships q2 = 2*q - 15 in int8 (a lossless, data-
independent relabeling of the 16 uint4 symbols). Dequant is then ONE DVE op
per ktile (w8 = q2 * s -> fp8) and the compensating 1/2 rides the a->fp8
conversion (power-of-two, exact: a8 = 0.5 * a). The fp16 ktiles rebuild
q-7.5 = 0.5*q2 on GpSimd, off the DVE critical path.

Device layouts:
 - aT[m_out, k_in, k_out*128 + m_in]: head (fp8 ktiles) transient, used for
   the A-matmuls + a8 conversion; tail (fp16 ktiles) resident as fp16 lhsT.
 - a8[mt] [128, NFP8, 128] fp8: [:, 2kp:2kp+2, :] slices are DoubleRow lhsT.
 - w8[kp] [128, 2, N_L] fp8: [:, :, nch] slices are DoubleRow moving operand.
 - scales broadcast to 128 partitions per kpair (only s; z never broadcasts).
 - mu is built on [32, N_L] then partition-stacked x4 so the correction
   matmuls for mtiles mt%4 = r run row-tiled at partition offset 32r.
"""

import numpy as np

M, K, N = 4096, 4096, 4096
G = 128          # quant group size
P = 128          # partitions
NCORES = 8
MG, NGRP = 4, 2           # core grid: 4 m-groups x 2 n-groups
ML = M // MG              # 1024 rows per core
NL = N // NGRP            # 2048 cols per core
MT_L = ML // P            # 8 m tiles per core
KT = K // P               # 32 k tiles (== quant groups)
NFP8 = 22                 # ktiles dequantized to fp8 (must be even)
KP8 = NFP8 // 2           # DoubleRow k-pairs
NCH = NL // 512           # 4 psum chunks of 512 cols
MBLK = 2                  # mtiles per psum block (MBLK*NCH = 8 banks)

_CACHE = {}


def _build_nc():
    import concourse.bacc as bacc
    import concourse.mybir as mybir
    import concourse.tile as tile
    from concourse.bass import ts

    f16 = mybir.dt.float16
    f32 = mybir.dt.float32
    i8 = mybir.dt.int8
    f8 = mybir.dt.float8e4
    DR = mybir.MatmulPerfMode.DoubleRow
    ALU = mybir.AluOpType

    HEADC = NFP8 * P          # 2816 head columns of aT (fp8 ktiles)
    TAILC = K - HEADC         # 1280 tail columns (fp16 ktiles)

    nc = bacc.Bacc("TRN2", target_bir_lowering=False, debug=False)

    aT = nc.dram_tensor("aT", [MT_L, P, K], f16, kind="ExternalInput").ap()
    q = nc.dram_tensor("q", [KT, P, NL], i8, kind="ExternalInput").ap()
    ssm = nc.dram_tensor("ssm", [1, KT * NL], f16, kind="ExternalInput").ap()
    sn = nc.dram_tensor("sn", [KT, NL], f16, kind="ExternalInput").ap()
    zn = nc.dram_tensor("zn", [KT, NL], f16, kind="ExternalInput").ap()
    out = nc.dram_tensor("out", [MT_L, P, NL], f32, kind="ExternalOutput").ap()

    with tile.TileContext(nc) as tc:
        with (
            tc.tile_pool(name="w8", bufs=KP8) as w8pool,
            tc.tile_pool(name="w16", bufs=KT - NFP8) as w16pool,
            tc.tile_pool(name="et", bufs=1) as etpool,
            tc.tile_pool(name="mu4", bufs=1) as mu4pool,
            tc.tile_pool(name="muz", bufs=2) as muzpool,
            tc.tile_pool(name="sbc", bufs=2) as sbcpool,
            tc.tile_pool(name="sbc1", bufs=2) as sbc1pool,
            tc.tile_pool(name="qt1", bufs=3) as q1pool,
            tc.tile_pool(name="d", bufs=2) as dpool,
            tc.tile_pool(name="ah", bufs=2) as ahpool,
            tc.tile_pool(name="atl", bufs=MT_L) as atlpool,
            tc.tile_pool(name="a8", bufs=MT_L) as a8pool,
            tc.tile_pool(name="a16q", bufs=2) as a16qpool,
            tc.tile_pool(name="ot", bufs=2) as opool,
            tc.tile_pool(name="ps", bufs=8, space="PSUM") as pspool,
        ):
            # PE warm-up: back-to-back matmuls on garbage pull the HAM clock
            # gate to 8/8 before real operands arrive.
            warm_in = dpool.tile([P, 512], f16, name="warm_in", tag="d")
            nc.gpsimd.memset(warm_in[:], 0.0)
            warm_ps = pspool.tile([P, 512], f32, name="warm_ps", tag="ps")
            for i in range(16):
                nc.tensor.matmul(
                    warm_ps[:],
                    warm_in[:, 0:P],
                    warm_in[:],
                    start=(i == 0),
                    stop=(i == 15),
                )

            # One-hot selector for the A matmuls: E[p, j] = 1 iff j == 31,
            # so E[:, 31-t : 63-t] is the [128, 32] matrix with column t ones.
            Et = etpool.tile([P, 63], f16, name="Et")
            nc.gpsimd.memset(Et[:], 0.0)
            nc.gpsimd.memset(Et[:, 31:32], 1.0)

            # mu[g, n] = (7.5 - z) * s on base-0 scratch (tensor_tensor needs
            # both SBUF inputs at equal base partition), then replicated to
            # partition offsets 0/32/64/96 for row-tiled corr matmuls.
            # zn/sn ride the scalar ring; the SBUF->SBUF stacking copies go on
            # the sync ring after the first aT heads so neither ring stalls.
            znt = muzpool.tile([KT, NL], f16, name="znt")
            nc.scalar.dma_start(znt[:], zn)
            snt = muzpool.tile([KT, NL], f16, name="snt")
            nc.scalar.dma_start(snt[:], sn)
            mut4 = mu4pool.tile([P, NL], f16, name="mut4")
            nc.vector.tensor_scalar(
                mut4[0:KT, :], znt[:], -1.0, 7.5, ALU.mult, ALU.add
            )
            nc.vector.tensor_mul(out=mut4[32:64, :], in0=mut4[0:KT, :], in1=snt[:])

            # ---- DMA-front: q loads (SWDGE) and s broadcasts split across
            # both HWDGE rings so neither serializes the dequant stream. ----
            qts, sbcs = [], []
            for t in range(KT):
                qt = q1pool.tile([P, NL], i8, tag="qt1", name=f"qt{t}")
                nc.gpsimd.dma_start(qt[:], q[t])
                qts.append(qt)
            ahs = []
            for mt in range(2):
                ah = ahpool.tile([P, HEADC], f16, name=f"ah{mt}", tag="ah")
                nc.sync.dma_start(ah[:], aT[mt][:, 0:HEADC])
                ahs.append(ah)
            nc.sync.dma_start(mut4[0:KT, :], mut4[32:64, :])
            for r in range(2, 4):
                nc.sync.dma_start(mut4[32 * r : 32 * (r + 1), :], mut4[32:64, :])
            for t in range(KT):
                pool = sbcpool if t % 2 == 0 else sbc1pool
                eng = nc.scalar if t % 2 == 0 else nc.sync
                sbc = pool.tile([P, NL], f16, tag=pool.name, name=f"sbc{t}")
                eng.dma_start(
                    sbc[:], ssm[:, t * NL : (t + 1) * NL].partition_broadcast(P)
                )
                sbcs.append(sbc)

            # ---- dequant: fp8 ktiles one DVE op each (q2 * s -> fp8);
            # fp16 ktiles on GpSimd (0.5*q2, then *s), off the DVE path. ----
            w8s, w16s = [], []
            for kp in range(KP8):
                w8 = w8pool.tile([P, 2, NL], f8, tag="w8")
                for j in (0, 1):
                    t = 2 * kp + j
                    nc.vector.tensor_mul(out=w8[:, j, :], in0=qts[t][:], in1=sbcs[t][:])
                w8s.append(w8)
            for t in range(NFP8, KT):
                d = dpool.tile([P, NL], f16, tag="d")
                nc.gpsimd.tensor_scalar_mul(d[:], qts[t][:], 0.5)
                w16 = w16pool.tile([P, NL], f16, tag="w16")
                nc.gpsimd.tensor_mul(out=w16[:], in0=d[:], in1=sbcs[t][:])
                w16s.append(w16)

            # ---- A-phase quad 0 (mtiles 0-3) + a8 conversions ----
            atails = [None] * MT_L
            a8s = [None] * MT_L
            at16qs = [None, None]

            def emit_aphase(mt):
                if mt < len(ahs):
                    ah = ahs[mt]
                else:
                    ah = ahpool.tile([P, HEADC], f16, name=f"ah{mt}", tag="ah")
                    nc.sync.dma_start(ah[:], aT[mt][:, 0:HEADC])
                atl = atlpool.tile([P, TAILC], f16, name=f"atl{mt}", tag="atl")
                nc.sync.dma_start(atl[:], aT[mt][:, HEADC:K])
                atails[mt] = atl
                # a8 = 0.5 * a (exact), fp8, only the fp8 ktile columns.
                a8 = a8pool.tile([P, NFP8, P], f8, name=f"a8_{mt}", tag="a8")
                nc.scalar.activation(
                    a8[:], ah[:], mybir.ActivationFunctionType.Copy, scale=0.5
                )
                a8s[mt] = a8
                # A^T[g, m] column-tiled: mtile mt -> psA quad mt//4, col 32*(mt%4).
                qd, r = divmod(mt, 4)
                if r == 0:
                    emit_aphase.psA = pspool.tile(
                        [P, 512], f32, tag="ps", name=f"psA{qd}"
                    )
                for t in range(KT):
                    src = ah[:, ts(t, P)] if t < NFP8 else atl[:, ts(t - NFP8, P)]
                    nc.tensor.matmul(
                        emit_aphase.psA[32 * r : 32 * (r + 1), 0:P],
                        Et[:, 31 - t : 63 - t],
                        src,
                        start=(t == 0),
                        stop=(t == KT - 1),
                        tile_position=(0, 32 * r),
                    )
                if r == 3:
                    a16 = a16qpool.tile([P, P], f16, tag="a16q", name=f"a16q{qd}")
                    nc.scalar.copy(a16[:], emit_aphase.psA[:, 0:P])
                    at16qs[qd] = a16

            for mt in range(4):
                emit_aphase(mt)

            # ---- main loop: blocks of MBLK mtiles x NCH chunks = 8 psums ----
            NT16 = KT - NFP8
            for blk in range(MT_L // MBLK):
                mts = range(blk * MBLK, (blk + 1) * MBLK)
                pss = {}
                # DoubleRow fp8 opens each psum group (kp-outer: the
                # stationary a8 slice reuses across the NCH streams).
                for kp in range(KP8):
                    for mt in mts:
                        for nch in range(NCH):
                            if kp == 0:
                                pss[(mt, nch)] = pspool.tile(
                                    [P, 512], f32, tag="ps", name=f"ps{mt}_{nch}"
                                )
                            nc.tensor.matmul(
                                pss[(mt, nch)][:],
                                a8s[mt][:, 2 * kp : 2 * kp + 2, :],
                                w8s[kp][:, :, ts(nch, 512)],
                                start=(kp == 0),
                                stop=False,
                                perf_mode=DR,
                            )
                # rank-32 correction, row-tiled at partition 32*(mt%4).
                for mt in mts:
                    qd, r = divmod(mt, 4)
                    for nch in range(NCH):
                        nc.tensor.matmul(
                            pss[(mt, nch)][:],
                            at16qs[qd][32 * r : 32 * (r + 1), :],
                            mut4[32 * r : 32 * (r + 1), ts(nch, 512)],
                            start=False,
                            stop=False,
                            tile_position=(32 * r, 0),
                        )
                # fp16 tail ktiles, t-inner so chunks close staggered and the
                # drains overlap the next chunk's matmuls.
                for mt in mts:
                    for nch in range(NCH):
                        for i in range(NT16):
                            nc.tensor.matmul(
                                pss[(mt, nch)][:],
                                atails[mt][:, ts(i, P)],
                                w16s[i][:, ts(nch, 512)],
                                start=False,
                                stop=(i == NT16 - 1),
                            )
                        ot = opool.tile([P, 512], f32, tag="ot")
                        nc.scalar.copy(ot[:], pss[(mt, nch)][:])
                        nc.scalar.dma_start(out[mt][:, ts(nch, 512)], ot[:])
                if blk == 0:
                    for mt in range(4, MT_L):
                        emit_aphase(mt)

    nc.compile()
    return nc


def _shard_inputs(a, q_weight, scales, zeros):
    """Host-side shard/layout: slicing, transposition, replication, and the
    lossless int8 re-encoding q2 = 2*q - 15 of the uint4 symbols."""
    # aT[m_out, k_in, k_out*128 + m_in] = a[m_out*128 + m_in, k_out*128 + k_in]
    aT = np.ascontiguousarray(
        a.reshape(M // P, P, KT, P).transpose(0, 3, 2, 1)
    ).reshape(M // P, P, K)
    q2 = (q_weight * 2 - 15).astype(np.int8)

    in_maps = []
    for c in range(NCORES):
        mg, ng = divmod(c, NGRP)
        sl = slice(ng * NL, (ng + 1) * NL)
        s_c = np.ascontiguousarray(scales[:, sl])
        z_c = np.ascontiguousarray(zeros[:, sl])
        in_maps.append(
            {
                "aT": aT[mg * MT_L : (mg + 1) * MT_L],
                "q": np.ascontiguousarray(q2[:, sl]).reshape(KT, P, NL),
                "ssm": s_c.reshape(1, KT * NL),
                "sn": s_c,
                "zn": z_c,
            }
        )
    return in_maps


def _run(inputs, trace=False):
    from concourse import bass_utils

    if "nc" not in _CACHE:
        _CACHE["nc"] = _build_nc()
    nc = _CACHE["nc"]

    a = np.asarray(inputs["a"], dtype=np.float16)
    q_weight = np.asarray(inputs["q_weight"], dtype=np.int32)
    scales = np.asarray(inputs["scales"], dtype=np.float16)
    zeros = np.asarray(inputs["zeros"], dtype=np.float16)

    in_maps = _shard_inputs(a, q_weight, scales, zeros)
    res = bass_utils.run_bass_kernel_spmd(
        nc, in_maps, core_ids=list(range(NCORES)), trace=trace
    )

    out = np.empty((M, N), dtype=np.float32)
    for c in range(NCORES):
        mg, ng = divmod(c, NGRP)
        out[mg * ML : (mg + 1) * ML, ng * NL : (ng + 1) * NL] = res.results[c][
            "out"
        ].reshape(ML, NL)
    return out, res


def kernel(**inputs) -> np.ndarray:
    out, _ = _run(inputs, trace=False)
    return out


# revision 14
# speedup vs baseline: 1.9708x; 1.9708x over previous
"""Quantized matmul (uint4 groupwise dequant) on 8 Trainium2 NeuronCores.

Computes out = a_f32 @ W where W[k, n] = (q[k, n] - zeros[k//128, n]) * scales[k//128, n].

Sharding: 2-D tensor-parallel (4 m-groups x 2 n-groups). Each core gets
M_L = 1024 rows of `a` and N_L = 2048 output columns of q/scales/zeros.
This is the min-DMA sharding (24.4 MB/core vs 42 MB for pure-N TP).

Algorithm (hybrid fp8 DoubleRow + fp16, all arithmetic on device):
  W = Wc + rep(mu), with Wc[k,n] = (q[k,n] - 7.5) * s[g,n]  (zero-mean-ish)
  and mu[g,n] = (7.5 - z[g,n]) * s[g,n].
  out = a @ Wc + A @ mu, where A[m,g] = sum_{k in group g} a[m,k].

  - ktiles 0..NFP8-1 of Wc go to fp8e4; a goes to fp8e4; those contractions
    run with perf_mode=DoubleRow (2 k-planes per pass). Centering by 7.5
    (not z) keeps E[Wc^2] low enough that the fp8 rounding noise of both
    operands stays inside the 2e-2 rel-err budget.
  - Remaining ktiles stay fp16 (exact inputs) to claw back precision.
  - The rank-32 correction A @ mu runs in fp16. A is built on the PE with
    one-hot selector matmuls (exact fp16 a), 4-way column-tiled so four
    mtiles' A columns compute concurrently.

Encoding trick: the host ships q2 = 2*q - 15 in int8 (a lossless, data-
independent relabeling of the 16 uint4 symbols). Dequant is then ONE DVE op
per ktile (w8 = q2 * s -> fp8) and the compensating 1/2 rides the a->fp8
conversion (power-of-two, exact: a8 = 0.5 * a). The fp16 ktiles rebuild
q-7.5 = 0.5*q2 on GpSimd, off the DVE critical path.

Device layouts:
 - aT[m_out, k_in, k_out*128 + m_in]: head (fp8 ktiles) transient, used for
   the A-matmuls + a8 conversion; tail (fp16 ktiles) resident as fp16 lhsT.
 - a8[mt] [128, NFP8, 128] fp8: [:, 2kp:2kp+2, :] slices are DoubleRow lhsT.
 - w8[kp] [128, 2, N_L] fp8: [:, :, nch] slices are DoubleRow moving operand.
 - scales broadcast to 128 partitions per kpair (only s; z never broadcasts).
 - mu is built on [32, N_L] then partition-stacked x4 so the correction
   matmuls for mtiles mt%4 = r run row-tiled at partition offset 32r.
"""

import numpy as np

M, K, N = 4096, 4096, 4096
G = 128          # quant group size
P = 128          # partitions
NCORES = 8
MG, NGRP = 4, 2           # core grid: 4 m-groups x 2 n-groups
ML = M // MG              # 1024 rows per core
NL = N // NGRP            # 2048 cols per core
MT_L = ML // P            # 8 m tiles per core
KT = K // P               # 32 k tiles (== quant groups)
NFP8 = 22                 # ktiles dequantized to fp8 (must be even)
KP8 = NFP8 // 2           # DoubleRow k-pairs
NCH = NL // 512           # 4 psum chunks of 512 cols
MBLK = 2                  # mtiles per psum block (MBLK*NCH = 8 banks)

_CACHE = {}


def _build_nc():
    import concourse.bacc as bacc
    import concourse.mybir as mybir
    import concourse.tile as tile
    from concourse.bass import ts

    f16 = mybir.dt.float16
    f32 = mybir.dt.float32
    i8 = mybir.dt.int8
    f8 = mybir.dt.float8e4
    DR = mybir.MatmulPerfMode.DoubleRow
    ALU = mybir.AluOpType

    HEADC = NFP8 * P          # 2816 head columns of aT (fp8 ktiles)
    TAILC = K - HEADC         # 1280 tail columns (fp16 ktiles)

    nc = bacc.Bacc("TRN2", target_bir_lowering=False, debug=False)

    aT = nc.dram_tensor("aT", [MT_L, P, K], f16, kind="ExternalInput").ap()
    q = nc.dram_tensor("q", [KT, P, NL], i8, kind="ExternalInput").ap()
    ssm = nc.dram_tensor("ssm", [1, KT * NL], f16, kind="ExternalInput").ap()
    sn = nc.dram_tensor("sn", [KT, NL], f16, kind="ExternalInput").ap()
    zn = nc.dram_tensor("zn", [KT, NL], f16, kind="ExternalInput").ap()
    out = nc.dram_tensor("out", [MT_L, P, NL], f32, kind="ExternalOutput").ap()

    with tile.TileContext(nc) as tc:
        with (
            tc.tile_pool(name="w8", bufs=KP8) as w8pool,
            tc.tile_pool(name="w16", bufs=KT - NFP8) as w16pool,
            tc.tile_pool(name="et", bufs=1) as etpool,
            tc.tile_pool(name="mu4", bufs=1) as mu4pool,
            tc.tile_pool(name="muz", bufs=2) as muzpool,
            tc.tile_pool(name="sbc", bufs=2) as sbcpool,
            tc.tile_pool(name="sbc1", bufs=2) as sbc1pool,
            tc.tile_pool(name="qt1", bufs=3) as q1pool,
            tc.tile_pool(name="dt", bufs=2) as dtpool,
            tc.tile_pool(name="ah", bufs=2) as ahpool,
            tc.tile_pool(name="atl", bufs=MT_L) as atlpool,
            tc.tile_pool(name="a8", bufs=MT_L) as a8pool,
            tc.tile_pool(name="a16q", bufs=2) as a16qpool,
            tc.tile_pool(name="ot", bufs=2) as opool,
            tc.tile_pool(name="ps", bufs=8, space="PSUM") as pspool,
        ):
            # PE warm-up: back-to-back matmuls on garbage pull the HAM clock
            # gate to 8/8 before real operands arrive.
            warm_in = dtpool.tile([P, 512], f16, name="warm_in", tag="dt")
            nc.gpsimd.memset(warm_in[:], 0.0)
            warm_ps = pspool.tile([P, 512], f32, name="warm_ps", tag="ps")
            for i in range(16):
                nc.tensor.matmul(
                    warm_ps[:],
                    warm_in[:, 0:P],
                    warm_in[:],
                    start=(i == 0),
                    stop=(i == 15),
                )

            # One-hot selector for the A matmuls: E[p, j] = 1 iff j == 31,
            # so E[:, 31-t : 63-t] is the [128, 32] matrix with column t ones.
            Et = etpool.tile([P, 63], f16, name="Et")
            nc.gpsimd.memset(Et[:], 0.0)
            nc.gpsimd.memset(Et[:, 31:32], 1.0)

            # mu[g, n] = (7.5 - z) * s on base-0 scratch (tensor_tensor needs
            # both SBUF inputs at equal base partition), then replicated to
            # partition offsets 0/32/64/96 for row-tiled corr matmuls.
            # zn/sn ride the scalar ring; the SBUF->SBUF stacking copies go on
            # the sync ring after the first aT heads so neither ring stalls.
            znt = muzpool.tile([KT, NL], f16, name="znt")
            nc.scalar.dma_start(znt[:], zn)
            snt = muzpool.tile([KT, NL], f16, name="snt")
            nc.scalar.dma_start(snt[:], sn)
            mut4 = mu4pool.tile([P, NL], f16, name="mut4")
            nc.vector.tensor_scalar(
                mut4[0:KT, :], znt[:], -1.0, 7.5, ALU.mult, ALU.add
            )
            nc.vector.tensor_mul(out=mut4[32:64, :], in0=mut4[0:KT, :], in1=snt[:])

            # ---- DMA-front: q loads (SWDGE) and s broadcasts split across
            # both HWDGE rings so neither serializes the dequant stream. ----
            qts, sbcs = [], []
            for t in range(KT):
                qt = q1pool.tile([P, NL], i8, tag="qt1", name=f"qt{t}")
                eng = nc.scalar if t % 2 == 0 else nc.sync
                eng.dma_start(qt[:], q[t])
                qts.append(qt)
            ahs = []
            for mt in range(2):
                ah = ahpool.tile([P, K], f16, name=f"ah{mt}", tag="ah")
                nc.sync.dma_start(ah[:], aT[mt])
                ahs.append(ah)
            nc.sync.dma_start(mut4[0:KT, :], mut4[32:64, :])
            for r in range(2, 4):
                nc.sync.dma_start(mut4[32 * r : 32 * (r + 1), :], mut4[32:64, :])
            for t in range(KT):
                pool = sbcpool if t % 2 == 0 else sbc1pool
                eng = nc.scalar if t % 2 == 0 else nc.sync
                sbc = pool.tile([P, NL], f16, tag=pool.name, name=f"sbc{t}")
                eng.dma_start(
                    sbc[:], ssm[:, t * NL : (t + 1) * NL].partition_broadcast(P)
                )
                sbcs.append(sbc)

            # ---- dequant: one DVE mul per ktile (q2 * s -> f16). fp8 tiles
            # then cast f16->f8 on ACT (even) / SWDGE cast-DMA (odd); fp16
            # tiles keep the f16 product (the 0.5 rides the fp16 lhsT). ----
            w8s, w16s = [], []
            for kp in range(KP8):
                w8 = w8pool.tile([P, 2, NL], f8, tag="w8")
                for j in (0, 1):
                    t = 2 * kp + j
                    dt = dtpool.tile([P, NL], f16, tag="dt")
                    nc.vector.tensor_mul(out=dt[:], in0=qts[t][:], in1=sbcs[t][:])
                    if kp % 2 == 0:
                        nc.scalar.copy(w8[:, j, :], dt[:])
                    else:
                        nc.gpsimd.dma_start(w8[:, j, :], dt[:])
                w8s.append(w8)
            for t in range(NFP8, KT):
                w16 = w16pool.tile([P, NL], f16, tag="w16")
                nc.vector.tensor_mul(out=w16[:], in0=qts[t][:], in1=sbcs[t][:])
                w16s.append(w16)

            # ---- A-phase quad 0 (mtiles 0-3) + a8 conversions ----
            atails = [None] * MT_L
            a8s = [None] * MT_L
            at16qs = [None, None]

            def emit_aphase(mt):
                if mt < len(ahs):
                    ah = ahs[mt]
                else:
                    ah = ahpool.tile([P, K], f16, name=f"ah{mt}", tag="ah")
                    nc.sync.dma_start(ah[:], aT[mt])
                # a8 = 0.5 * a (exact power-of-two), fp8, fp8-ktile columns.
                a8 = a8pool.tile([P, NFP8, P], f8, name=f"a8_{mt}", tag="a8")
                nc.scalar.activation(
                    a8[:], ah[:, 0:HEADC], mybir.ActivationFunctionType.Copy,
                    scale=0.5,
                )
                a8s[mt] = a8
                # fp16 lhsT = 0.5 * a tail (exact); pairs with w16 = q2*s.
                atl = atlpool.tile([P, TAILC], f16, name=f"atl{mt}", tag="atl")
                nc.scalar.activation(
                    atl[:], ah[:, HEADC:K], mybir.ActivationFunctionType.Copy,
                    scale=0.5,
                )
                atails[mt] = atl
                # A^T[g, m] column-tiled: mtile mt -> psA quad mt//4, col 32*(mt%4).
                qd, r = divmod(mt, 4)
                if r == 0:
                    emit_aphase.psA = pspool.tile(
                        [P, 512], f32, tag="ps", name=f"psA{qd}"
                    )
                for t in range(KT):
                    nc.tensor.matmul(
                        emit_aphase.psA[32 * r : 32 * (r + 1), 0:P],
                        Et[:, 31 - t : 63 - t],
                        ah[:, ts(t, P)],
                        start=(t == 0),
                        stop=(t == KT - 1),
                        tile_position=(0, 32 * r),
                    )
                if r == 3:
                    a16 = a16qpool.tile([P, P], f16, tag="a16q", name=f"a16q{qd}")
                    nc.scalar.copy(a16[:], emit_aphase.psA[:, 0:P])
                    at16qs[qd] = a16

            for mt in range(4):
                emit_aphase(mt)

            # ---- main loop: blocks of MBLK mtiles x NCH chunks = 8 psums ----
            NT16 = KT - NFP8
            for blk in range(MT_L // MBLK):
                mts = range(blk * MBLK, (blk + 1) * MBLK)
                pss = {}
                # DoubleRow fp8 opens each psum group (kp-outer: the
                # stationary a8 slice reuses across the NCH streams).
                for kp in range(KP8):
                    for mt in mts:
                        for nch in range(NCH):
                            if kp == 0:
                                pss[(mt, nch)] = pspool.tile(
                                    [P, 512], f32, tag="ps", name=f"ps{mt}_{nch}"
                                )
                            nc.tensor.matmul(
                                pss[(mt, nch)][:],
                                a8s[mt][:, 2 * kp : 2 * kp + 2, :],
                                w8s[kp][:, :, ts(nch, 512)],
                                start=(kp == 0),
                                stop=False,
                                perf_mode=DR,
                            )
                # rank-32 correction, row-tiled at partition 32*(mt%4).
                for mt in mts:
                    qd, r = divmod(mt, 4)
                    for nch in range(NCH):
                        nc.tensor.matmul(
                            pss[(mt, nch)][:],
                            at16qs[qd][32 * r : 32 * (r + 1), :],
                            mut4[32 * r : 32 * (r + 1), ts(nch, 512)],
                            start=False,
                            stop=False,
                            tile_position=(32 * r, 0),
                        )
                # fp16 tail ktiles, t-inner so chunks close staggered and the
                # drains overlap the next chunk's matmuls.
                for mt in mts:
                    for nch in range(NCH):
                        for i in range(NT16):
                            nc.tensor.matmul(
                                pss[(mt, nch)][:],
                                atails[mt][:, ts(i, P)],
                                w16s[i][:, ts(nch, 512)],
                                start=False,
                                stop=(i == NT16 - 1),
                            )
                        ot = opool.tile([P, 512], f32, tag="ot")
                        nc.scalar.copy(ot[:], pss[(mt, nch)][:])
                        nc.scalar.dma_start(out[mt][:, ts(nch, 512)], ot[:])
                if blk == 0:
                    for mt in range(4, MT_L):
                        emit_aphase(mt)

    nc.compile()
    return nc


def _shard_inputs(a, q_weight, scales, zeros):
    """Host-side shard/layout: slicing, transposition, replication, and the
    lossless int8 re-encoding q2 = 2*q - 15 of the uint4 symbols."""
    # aT[m_out, k_in, k_out*128 + m_in] = a[m_out*128 + m_in, k_out*128 + k_in]
    aT = np.ascontiguousarray(
        a.reshape(M // P, P, KT, P).transpose(0, 3, 2, 1)
    ).reshape(M // P, P, K)
    q2 = (q_weight * 2 - 15).astype(np.int8)

    in_maps = []
    for c in range(NCORES):
        mg, ng = divmod(c, NGRP)
        sl = slice(ng * NL, (ng + 1) * NL)
        s_c = np.ascontiguousarray(scales[:, sl])
        z_c = np.ascontiguousarray(zeros[:, sl])
        in_maps.append(
            {
                "aT": aT[mg * MT_L : (mg + 1) * MT_L],
                "q": np.ascontiguousarray(q2[:, sl]).reshape(KT, P, NL),
                "ssm": s_c.reshape(1, KT * NL),
                "sn": s_c,
                "zn": z_c,
            }
        )
    return in_maps


def _run(inputs, trace=False):
    from concourse import bass_utils

    if "nc" not in _CACHE:
        _CACHE["nc"] = _build_nc()
    nc = _CACHE["nc"]

    a = np.asarray(inputs["a"], dtype=np.float16)
    q_weight = np.asarray(inputs["q_weight"], dtype=np.int32)
    scales = np.asarray(inputs["scales"], dtype=np.float16)
    zeros = np.asarray(inputs["zeros"], dtype=np.float16)

    in_maps = _shard_inputs(a, q_weight, scales, zeros)
    res = bass_utils.run_bass_kernel_spmd(
        nc, in_maps, core_ids=list(range(NCORES)), trace=trace
    )

    out = np.empty((M, N), dtype=np.float32)
    for c in range(NCORES):
        mg, ng = divmod(c, NGRP)
        out[mg * ML : (mg + 1) * ML, ng * NL : (ng + 1) * NL] = res.results[c][
            "out"
        ].reshape(ML, NL)
    return out, res


def kernel(**inputs) -> np.ndarray:
    out, _ = _run(inputs, trace=False)
    return out
